# revision 66
# baseline (speedup 1.0000x reference)
"""Trainium2 Bass kernel for packed-sequence GQA attention (nn_Attention_84602265796942).

Sharding: data-parallel over the B=16 packed sequences -> 2 sequences (1024
tokens) per core, weights replicated. Zero collectives.

Per-core pipeline (all matmuls bf16 x bf16 -> fp32 PSUM):
  A) QKV projections from host-transposed inputs; RoPE applied via a
     host-side head-dim permutation ([a0..a15 b0..b15 a16..a31 b16..b31])
     so the rotation partner sits at partition r^16 (one stream_shuffle).
  B) Attention per (block, head-pair), emitted interleaved with the Q
     projection so PE/ACT/DVE overlap across phases: q-heads are
     host-permuted so pair (h, h+4) shares a qt tile and maps to kv heads
     (2g, 2g+1) = the two partition halves of one K tile; scores are
     computed transposed (scoresT[m,l]) with paired K=64 matmuls at
     partition bases 0/64 (concurrent PE row groups); softmax without max
     subtraction (scores are bounded); probs row-sums obtained via a
     ones-column appended to V in the PV matmul (M=65); normalization
     deferred to a per-column scale of the attention output.
  C) Output projection from the transposed attention output.

PSUM pools are sized 4+2+2 banks so projection, score, and PV
accumulations coexist within the 8 banks and the phases pipeline; every
PSUM tile has a single reader that stages it to SBUF so banks release
as early as possible.
"""
import numpy as np
import ml_dtypes

import concourse.bass as bass
import concourse.tile as tile
from concourse import bacc, mybir
from concourse.bass_utils import run_bass_kernel_spmd

F32 = mybir.dt.float32
BF16 = mybir.dt.bfloat16

B, L, DIM, H, HKV, DH = 16, 512, 2048, 32, 8, 64
REP = H // HKV
S = B * L
NCORE = 8
S_LOC = S // NCORE          # 1024 tokens per core
NBLK = S_LOC // L           # 2 blocks per core
SCALE = DH ** -0.5

# within-head dim permutation: rows [a0..a15, b0..b15, a16..a31, b16..b31]
PERM64 = np.concatenate([np.arange(0, 32, 2), np.arange(1, 32, 2),
                         np.arange(32, 64, 2), np.arange(33, 64, 2)])
_rr = np.arange(64)
FREQ_IDX = (_rr // 32) * 16 + (_rr % 16)
C2_SIGN = np.where((_rr % 32) < 16, -1.0, 1.0).astype(np.float32)
# q-head order: pair (h, h+4) within each group of 8 -> kv heads (2g, 2g+1)
HPERM = np.array([8 * gi + t + 4 * half
                  for gi in range(4) for t in range(4) for half in range(2)])

_CACHED = {}

LAST_RESULTS = None  # BassKernelResults of the most recent run (for test.py)


def _build():
    nc = bacc.Bacc("TRN2", target_bir_lowering=False, debug=False,
                   num_devices=NCORE)

    xT_d = nc.dram_tensor("xT", [DIM, S_LOC], BF16, kind="ExternalInput")
    wqT_d = nc.dram_tensor("wqT", [DIM, H * DH], BF16, kind="ExternalInput")
    wkT_d = nc.dram_tensor("wkT", [DIM, HKV * DH], BF16, kind="ExternalInput")
    wvT_d = nc.dram_tensor("wvT", [DIM, HKV * DH], BF16, kind="ExternalInput")
    woT_d = nc.dram_tensor("woT", [H * DH, DIM], BF16, kind="ExternalInput")
    c1_d = nc.dram_tensor("c1", [128, S_LOC], BF16, kind="ExternalInput")
    c2_d = nc.dram_tensor("c2", [128, S_LOC], BF16, kind="ExternalInput")
    out_d = nc.dram_tensor("out", [S_LOC, DIM], F32, kind="ExternalOutput")

    KD = DIM // 128          # 16 contraction tiles
    NQI = (H * DH) // 128    # 16 Q row-tiles (one head pair each)
    NKI = (HKV * DH) // 128  # 4 K row-tiles
    NMT = L // 128           # 4 token tiles per block
    EXP = mybir.ActivationFunctionType.Exp
    SHUF_MASK = [i ^ 16 for i in range(32)]

    with tile.TileContext(nc) as tc:
        with (
            tc.tile_pool(name="persist", bufs=1) as pp,      # long-lived activations
            tc.tile_pool(name="scratch", bufs=2) as sp,      # rope/norm scratch
        ):
            # persistent activation tensors
            qt = [[pp.tile([128, L], BF16, tag=f"qt{i}_{b}", name=f"qt{i}_{b}")
                   for b in range(NBLK)] for i in range(NQI)]
            kt = [[pp.tile([128, L], BF16, tag=f"kt{g}_{b}", name=f"kt{g}_{b}")
                   for b in range(NBLK)] for g in range(NKI)]
            vaug = [pp.tile([128, HKV * (DH + 1)], BF16, tag=f"va{m}", name=f"va{m}")
                    for m in range(S_LOC // 128)]
            att = [[pp.tile([128, L], BF16, tag=f"at{i}_{b}", name=f"at{i}_{b}")
                    for b in range(NBLK)] for i in range(NQI)]

            # ================= phase A: QKV projections + RoPE =================
            with (
                tc.tile_pool(name="inA", bufs=1) as pa,
                tc.tile_pool(name="wslab", bufs=1) as wp,
                tc.tile_pool(name="psA", bufs=4, space="PSUM") as psA,
                tc.tile_pool(name="probs", bufs=5) as probp,
                tc.tile_pool(name="psS", bufs=2, space="PSUM") as psS,
                tc.tile_pool(name="psO", bufs=2, space="PSUM") as psO,
            ):
                def rope_epilogue(ps, b, dst128):
                    """ps: [128, 512] psum of pre-rope QT/KT rows -> bf16 dst."""
                    cs = slice(b * L, (b + 1) * L)
                    qf = sp.tile([128, L], F32, tag="qf", name="qf")
                    nc.vector.tensor_copy(qf[:], ps[:])   # single PSUM reader
                    sh = sp.tile([128, L], F32, tag="sh", name="sh")
                    nc.vector.stream_shuffle(sh[:], qf[:], SHUF_MASK)
                    t1 = sp.tile([128, L], BF16, tag="t1", name="t1")
                    nc.vector.tensor_mul(t1[:], qf[:], c1[:, cs])
                    t2 = sp.tile([128, L], BF16, tag="t2", name="t2")
                    nc.vector.tensor_mul(t2[:], sh[:], c2[:, cs])
                    nc.vector.tensor_add(dst128[:], t1[:], t2[:])

                xT, wkT, wvT = [], [], []
                for k in range(KD):
                    t = pa.tile([128, S_LOC], BF16, tag=f"xT{k}", name=f"xT{k}")
                    nc.sync.dma_start(t[:], xT_d[k * 128:(k + 1) * 128, :])
                    xT.append(t)
                    t = pa.tile([128, HKV * DH], BF16, tag=f"wvT{k}", name=f"wvT{k}")
                    nc.sync.dma_start(t[:], wvT_d[k * 128:(k + 1) * 128, :])
                    wvT.append(t)
                for k in range(KD):
                    t = pa.tile([128, HKV * DH], BF16, tag=f"wkT{k}", name=f"wkT{k}")
                    nc.sync.dma_start(t[:], wkT_d[k * 128:(k + 1) * 128, :])
                    wkT.append(t)
                c1 = pa.tile([128, S_LOC], BF16, tag="c1", name="c1s")
                c2 = pa.tile([128, S_LOC], BF16, tag="c2", name="c2s")
                nc.sync.dma_start(c1[:], c1_d[:])
                nc.sync.dma_start(c2[:], c2_d[:])

                # ---- V projection ----
                for m in range(S_LOC // 128):
                    vm = psA.tile([128, HKV * DH], F32, tag="aps", name="aps")
                    for k in range(KD):
                        nc.tensor.matmul(
                            vm[:], xT[k][:, m * 128:(m + 1) * 128], wvT[k][:],
                            start=(k == 0), stop=(k == KD - 1))
                    nc.vector.memset(vaug[m][:], 1.0)
                    nc.vector.tensor_copy(
                        vaug[m].rearrange("p (g d) -> p g d", d=DH + 1)[:, :, 0:DH],
                        vm.rearrange("p (g d) -> p g d", d=DH))

                # ---- K projection + rope ----
                for i in range(NKI):
                    for b in range(NBLK):
                        ps = psA.tile([128, L], F32, tag="aps", name="aps")
                        for k in range(KD):
                            nc.tensor.matmul(
                                ps[:], wkT[k][:, i * 128:(i + 1) * 128],
                                xT[k][:, b * L:(b + 1) * L],
                                start=(k == 0), stop=(k == KD - 1))
                        rope_epilogue(ps, b, kt[i][b])

                def attention(hp, b):
                    """scores/softmax/PV/normalize for head pair hp, block b."""
                    gi = hp // 4
                    probs_e, probs_o = [], []
                    for mi in range(NMT):
                        se = psS.tile([128, L], F32, tag="s", name="sps")
                        nc.tensor.matmul(
                            se[:],
                            kt[gi][b][0:64, mi * 128:(mi + 1) * 128],
                            qt[hp][b][0:64, :])
                        so = psS.tile([128, L], F32, tag="s", name="sps")
                        nc.tensor.matmul(
                            so[:],
                            kt[gi][b][64:128, mi * 128:(mi + 1) * 128],
                            qt[hp][b][64:128, :])
                        pe = probp.tile([128, L], BF16, tag="pe", name="pe")
                        nc.scalar.activation(pe[:], se[:], EXP, scale=SCALE)
                        po = probp.tile([128, L], BF16, tag="po", name="po")
                        nc.scalar.activation(po[:], so[:], EXP, scale=SCALE)
                        probs_e.append(pe)
                        probs_o.append(po)
                    oe = psO.tile([DH + 1, L], F32, tag="o", name="ops_o")
                    oo = psO.tile([DH + 1, L], F32, tag="o", name="ops_o")
                    ge, go = 2 * gi, 2 * gi + 1
                    for mi in range(NMT):
                        vm = vaug[b * NMT + mi]
                        nc.tensor.matmul(
                            oe[:], vm[:, ge * (DH + 1):(ge + 1) * (DH + 1)],
                            probs_e[mi][:],
                            start=(mi == 0), stop=(mi == NMT - 1))
                        nc.tensor.matmul(
                            oo[:], vm[:, go * (DH + 1):(go + 1) * (DH + 1)],
                            probs_o[mi][:],
                            start=(mi == 0), stop=(mi == NMT - 1))
                    for o_ps, half in ((oe, slice(0, 64)), (oo, slice(64, 128))):
                        osb = sp.tile([DH + 1, L], F32, tag="osb", name="osb")
                        nc.vector.tensor_copy(osb[:], o_ps[:])  # frees psO bank early
                        rc = sp.tile([1, L], F32, tag="rc", name="rc")
                        nc.vector.reciprocal(rc[:], osb[DH:DH + 1, :])
                        bc = sp.tile([64, L], F32, tag="bc", name="bc")
                        nc.gpsimd.partition_broadcast(bc[:], rc[0:1, :])
                        nc.vector.tensor_mul(att[hp][b][half, :],
                                             osb[0:DH, :], bc[:])

                # ---- Q projection + rope + attention, interleaved ----
                for ig in range(4):
                    slab = []
                    for k in range(KD):
                        t = wp.tile([128, 512], BF16, tag=f"wq{k}", name=f"wq{k}")
                        nc.sync.dma_start(
                            t[:], wqT_d[k * 128:(k + 1) * 128,
                                        ig * 512:(ig + 1) * 512])
                        slab.append(t)
                    for ii in range(4):
                        i = ig * 4 + ii
                        for b in range(NBLK):
                            ps = psA.tile([128, L], F32, tag="aps", name="aps")
                            for k in range(KD):
                                nc.tensor.matmul(
                                    ps[:], slab[k][:, ii * 128:(ii + 1) * 128],
                                    xT[k][:, b * L:(b + 1) * L],
                                    start=(k == 0), stop=(k == KD - 1))
                            rope_epilogue(ps, b, qt[i][b])
                            attention(i, b)

            # ================= phase C: output projection =================
            with (
                tc.tile_pool(name="inC", bufs=1) as pc,
                tc.tile_pool(name="outsb", bufs=4) as op,
                tc.tile_pool(name="psC", bufs=4, space="PSUM") as psC,
            ):
                woT = []
                for k in range(NQI):
                    t = pc.tile([128, DIM], BF16, tag=f"woT{k}", name=f"woT{k}")
                    nc.sync.dma_start(t[:], woT_d[k * 128:(k + 1) * 128, :])
                    woT.append(t)
                for b in range(NBLK):
                    for st in range(NMT):           # token tile within block
                        for ec in range(DIM // 512):
                            ps = psC.tile([128, 512], F32, tag="ops", name="ops_c")
                            for k in range(NQI):
                                nc.tensor.matmul(
                                    ps[:],
                                    att[k][b][:, st * 128:(st + 1) * 128],
                                    woT[k][:, ec * 512:(ec + 1) * 512],
                                    start=(k == 0), stop=(k == NQI - 1))
                            ot = op.tile([128, 512], F32, tag="ot", name="ot")
                            nc.vector.tensor_copy(ot[:], ps[:])
                            nc.sync.dma_start(
                                out_d[(b * L + st * 128):(b * L + (st + 1) * 128),
                                      ec * 512:(ec + 1) * 512],
                                ot[:])

    nc.compile()
    return nc


def _prep_shared(wq, wk, wv, wo):
    bf = ml_dtypes.bfloat16

    # wq: head order HPERM, PERM64 within head
    wq_p = wq.reshape(H, DH, DIM)[HPERM][:, PERM64, :].reshape(H * DH, DIM)
    # wk: natural head order, PERM64 within head
    wk_p = wk.reshape(HKV, DH, DIM)[:, PERM64, :].reshape(HKV * DH, DIM)
    # wo columns: head order HPERM, dims unpermuted (V is not roped)
    wo_p = wo.reshape(DIM, H, DH)[:, HPERM, :].reshape(DIM, H * DH)

    wqT = np.ascontiguousarray(wq_p.T.astype(bf))
    wkT = np.ascontiguousarray(wk_p.T.astype(bf))
    wvT = np.ascontiguousarray(wv.T.astype(bf))
    woT = np.ascontiguousarray(wo_p.T.astype(bf))
    return wqT, wkT, wvT, woT


def kernel(x, freqs_cos, freqs_sin, wq, wk, wv, wo):
    global LAST_RESULTS
    x = np.asarray(x, np.float32)
    freqs_cos = np.asarray(freqs_cos, np.float32)
    freqs_sin = np.asarray(freqs_sin, np.float32)
    bf = ml_dtypes.bfloat16

    if "nc" not in _CACHED:
        _CACHED["nc"] = _build()
    nc = _CACHED["nc"]

    wqT, wkT, wvT, woT = _prep_shared(
        np.asarray(wq, np.float32), np.asarray(wk, np.float32),
        np.asarray(wv, np.float32), np.asarray(wo, np.float32))

    in_maps = []
    for c in range(NCORE):
        rows = slice(c * S_LOC, (c + 1) * S_LOC)
        xT = np.ascontiguousarray(x[rows].T.astype(bf))
        fcc = freqs_cos[rows]      # [S_LOC, 32]
        fss = freqs_sin[rows]
        c1h = fcc[:, FREQ_IDX].T   # [64, S_LOC]
        c2h = (fss[:, FREQ_IDX] * C2_SIGN[None, :]).T
        c1 = np.ascontiguousarray(np.concatenate([c1h, c1h], 0).astype(bf))
        c2 = np.ascontiguousarray(np.concatenate([c2h, c2h], 0).astype(bf))
        in_maps.append({"xT": xT, "wqT": wqT, "wkT": wkT, "wvT": wvT,
                        "woT": woT, "c1": c1, "c2": c2})

    res = None
    for attempt in range(3):
        try:
            res = run_bass_kernel_spmd(nc, in_maps, list(range(NCORE)))
            break
        except Exception:
            if attempt == 2:
                raise
            import time
            time.sleep(10)   # transient NRT device errors usually clear on retry
    LAST_RESULTS = res
    out = np.concatenate([res.results[c]["out"] for c in range(NCORE)], axis=0)
    return np.ascontiguousarray(out.astype(np.float32))
